# revision 43
# baseline (speedup 1.0000x reference)
"""Trainium2 Bass kernel for nn_BitSwapWrapper.

Reference computation:
    g    = x[rows, idx]                       # one gathered element per row
    u    = coeff * (bitflip(g, bit_pos) - g)
    pert = scatter(zeros_like(x), (rows, idx), u)
    out  = (x + pert) @ W + b

Because pert has exactly one nonzero per row, (x + pert) @ W decomposes as
    out[i, :] = (x @ W)[i, :] + u[i] * W[idx[i], :] + b
so no [B, F] scatter tensor is ever materialized: the kernel streams x
through a K-accumulated matmul and applies the rank-per-row correction with
an indirect-DMA gather of the needed W rows.

Distribution: 2D sharding — batch split in 2 halves x contraction (F) split
in 4 shards.  Core c = h*4 + f streams x[half h, shard f].T @ W[shard f]
into PSUM (bf16 operands, fp32 accumulate, W-stationary so the moving
operand is the 2048-wide batch), writes its partial [O, 2048] in bf16, and
separately computes the scatter correction u[i] * W[idx[i], :] for its own
512-row slice of the batch (exact fp32 u from host-gathered g, indirect-DMA
gather of W rows).  The host sums the 4 partials per batch half (fp32),
adds the per-row corrections at their global rows, and adds the bias.

bf16 operands halve the dominant HBM traffic vs fp32/fp32r; splitting W
over 4 cores cuts the replicated-W read 4x.  Per-core HBM bytes:
16.8 MB (x) + 2.1 MB (W shard) + 1.0 MB (partial out) + 0.6 MB (gathers +
corr) ~= 20.5 MB, vs 48.5 MB for pure data-parallel fp32.  Single-chunk
DMA slabs keep the tensor engine continuously fed (its p-state ramp rewards
gap-free execution) while the prep/gather/corr side traffic is front-loaded
into the PE warm-up window.
"""

import numpy as np

import concourse.bass as bass
import concourse.mybir as mybir
from concourse.bass_utils import run_bass_kernel_spmd
from concourse.tile import TileContext

N_CORES = 8
B, F, O = 4096, 16384, 256
BH, FS = 2, 4            # batch-halves x F-shards
BC = B // BH             # 2048 batch rows per core's matmul
Fs = F // FS             # 4096 contraction rows per core
P = 128
KCs = Fs // P            # 32 contraction chunks per core
CR = B // N_CORES        # 512 correction rows per core
CB = CR // P             # 4 correction row-blocks
NJ = BC // 512           # 4 moving-operand column chunks (512 = 1 PSUM bank)

F32 = mybir.dt.float32
BF16 = mybir.dt.bfloat16
I32 = mybir.dt.int32


def _split_multi_waits(nc):
    """This container's walrus build rejects more than one sync-wait command
    per instruction; split extras onto single-wait NOPs on the same engine."""
    cur_bb = nc.cur_bb.bb
    for f in nc.m.functions:
        for bb in f.blocks:
            il = bb.instructions
            i = 0
            while i < len(il):
                ins = il[i]
                si = getattr(ins, "sync_info", None)
                if si is not None and si.on_wait and len(si.on_wait) > 1:
                    waits = list(si.on_wait)
                    extra, keep = waits[:-1], waits[-1:]
                    carriers = []
                    for w in extra:
                        nop = nc.engines[ins.engine].nop(nofuse=True).ins
                        tail = cur_bb.instructions.pop()
                        assert tail is nop
                        nop.sync_info = mybir.SyncInfo(on_wait=[w], on_update=[])
                        carriers.append(nop)
                    ins.sync_info = mybir.SyncInfo(
                        on_wait=keep, on_update=list(si.on_update or [])
                    )
                    il[i:i] = carriers
                    i += len(carriers)
                i += 1


def build(reps=1, cpg=1, stream_bufs=8, with_bias=False, defer_ms=0.050, wcpg=1, ws0_pool=True, x0_pool=False):
    nc = bass.Bass("TRN2", target_bir_lowering=False, debug=False)
    xt = nc.dram_tensor("xt", [Fs, BC], BF16, kind="ExternalInput").ap()
    w = nc.dram_tensor("w", [F, O], BF16, kind="ExternalInput").ap()
    wsh = nc.dram_tensor("wsh", [Fs, O], BF16, kind="ExternalInput").ap()
    gh = nc.dram_tensor("gh", [P, CB], F32, kind="ExternalInput").ap()
    idx = nc.dram_tensor("idx", [P, CB], I32, kind="ExternalInput").ap()
    bpos = nc.dram_tensor("bpos", [P, CB], I32, kind="ExternalInput").ap()
    coeff = nc.dram_tensor("coeff", [P, 1], F32, kind="ExternalInput").ap()
    out = nc.dram_tensor("out", [O, BC], BF16, kind="ExternalOutput").ap()
    corr_o = nc.dram_tensor("corr", [CR, O], BF16, kind="ExternalOutput").ap()

    with TileContext(nc) as tc:
        with (
            tc.tile_pool(name="stream", bufs=stream_bufs) as stream,
            tc.tile_pool(name="consts", bufs=1) as consts,
            tc.tile_pool(name="epi", bufs=1) as epi,
            tc.tile_pool(name="psum", bufs=1, space="PSUM") as psum,
        ):
            ones_i = consts.tile([P, 1], I32, name="ones_i")
            nc.vector.memset(ones_i[:], 1)
            coeff_b = consts.tile([P, 1], F32, name="coeff_b")
            nc.gpsimd.dma_start(out=coeff_b[:], in_=coeff[:])

            for _ in range(reps):
                psums = [
                    [psum.tile([P, 512], F32, tag=f"ps{h}_{j}", name=f"ps{h}_{j}")
                     for j in range(NJ)]
                    for h in range(O // P)
                ]
                us = []

                def emit_u(m, bpt, g):
                    # u = coeff * (bitflip(g) - g) for row-block m: tiny DVE
                    # work, runs early while the DVE is otherwise idle
                    mask = epi.tile([P, 1], I32, tag=f"mask{m}", name=f"mask{m}")
                    nc.vector.tensor_scalar(
                        mask[:], ones_i[:], bpt[:, m:m + 1], None,
                        mybir.AluOpType.logical_shift_left,
                    )
                    gflip = epi.tile([P, 1], I32, tag=f"gflip{m}", name=f"gflip{m}")
                    nc.vector.tensor_tensor(
                        out=gflip[:], in0=g[:, m:m + 1].bitcast(I32), in1=mask[:],
                        op=mybir.AluOpType.bitwise_xor,
                    )
                    u = epi.tile([P, 1], F32, tag=f"u{m}", name=f"u{m}")
                    nc.vector.tensor_tensor(
                        out=u[:], in0=gflip[:].bitcast(F32), in1=g[:, m:m + 1],
                        op=mybir.AluOpType.subtract,
                    )
                    nc.vector.tensor_tensor(
                        out=u[:], in0=u[:], in1=coeff_b[:],
                        op=mybir.AluOpType.mult,
                    )
                    us.append(u)

                # chunk 0 arrives piecewise (per-bank x pieces) so the PE can
                # start after ~128 KB, and its matmuls are split N=128 so the
                # PE crosses its p-state ramp window on cheap instructions
                # instead of full-width ones priced at the cold clock
                ws0 = epi.tile([P, O], BF16, tag="ws0", name="ws0")
                with tc.high_priority():
                    # the first Ldweights needs ws0; the Pool/SWDGE ring is
                    # idle at program start so it lands ~1.5us earlier than
                    # behind the xs0 pieces on HWDGE
                    eng0 = nc.gpsimd if ws0_pool else nc.scalar
                    eng0.dma_start(out=ws0[:], in_=wsh[0:P, :])
                # chunk 0 arrives as per-bank pieces (finer arrival
                # granularity); optionally the first two ride the SWDGE
                # ring, whose first transfer launches ~1.2us before HWDGE's
                pieces = {}
                for j in range(NJ):
                    tj = stream.tile([P, 512], BF16, tag=f"xp0_{j}",
                                     name=f"xp0_{j}")
                    eng = nc.gpsimd if (x0_pool and j < 2) else nc.sync
                    eng.dma_start(
                        out=tj[:], in_=xt[0:P, j * 512:(j + 1) * 512])
                    pieces[(0, j)] = tj
                for j in range(NJ):
                    for h in range(O // P):
                        nc.tensor.matmul(
                            psums[h][j][:],
                            lhsT=ws0[:, h * P:(h + 1) * P],
                            rhs=pieces[(0, j)][:],
                            start=True,
                            stop=False,
                        )

                CPG = cpg  # k-chunks per DMA slab
                slabs = [(1 + i * CPG, CPG) for i in range((KCs - 1) // CPG)]
                rem = (KCs - 1) % CPG
                if rem:
                    slabs.append((KCs - rem, rem))

                # the W stream runs on its own coarser cadence (4-chunk
                # batches on the scalar ring): fewer ring events -> less
                # arrival jitter against the chunk-per-slab x stream
                WCPG = wcpg
                ws_of = {}
                wbatches = []
                kw = 1
                while kw < KCs:
                    wn = min(WCPG, KCs - kw)
                    wbatches.append((kw, wn))
                    kw += wn
                last_xs = None
                for k4, (k0, nch) in enumerate(slabs):
                    r0 = k0 * P
                    xs = stream.tile([P, nch * BC], BF16, tag="xs",
                                     name="xs", padded_shape=[P, CPG * BC])
                    last_xs = xs
                    for kw, wn in wbatches:
                        if kw != k0:
                            continue
                        wsb = stream.tile([P, wn * O], BF16, tag="wsb",
                                          name="wsb", padded_shape=[P, WCPG * O])
                        nc.scalar.dma_start(
                            out=wsb[:].rearrange("p (c o) -> p c o", c=wn),
                            in_=wsh[kw * P:(kw + wn) * P, :].rearrange(
                                "(c p) o -> p c o", p=P),
                        )
                        for c in range(wn):
                            ws_of[kw + c] = (wsb, c * O)
                    nc.sync.dma_start(
                        out=xs[:].rearrange("p (c b) -> p c b", c=nch),
                        in_=xt[r0:r0 + nch * P, :].rearrange(
                            "(c p) b -> p c b", p=P),
                    )
                    for c in range(nch):
                        last = k4 == len(slabs) - 1 and c == nch - 1
                        ws, wof = ws_of[k0 + c]
                        # the first streamed chunk accumulates in N=128
                        # pieces: cold-p-state-priced instructions then
                        # cover 1/4 the work while the PE clock ramps
                        # (start=False only accumulates, so safe for PSUM)
                        npc = 4 if k4 == 0 else 1
                        pw = 512 // npc
                        for h in range(O // P):
                            for j in range(NJ):
                                for piece in range(npc):
                                    b0 = c * BC + j * 512 + piece * pw
                                    nc.tensor.matmul(
                                        psums[h][j][:, piece * pw:(piece + 1) * pw],
                                        lhsT=ws[:, wof + h * P:wof + (h + 1) * P],
                                        rhs=xs[:, b0:b0 + pw],
                                        start=False,
                                        stop=last,
                                        skip_group_check=npc > 1,
                                    )
                assert slabs, "need at least one streamed slab"
                # Defer the W-row gathers + correction writes to the tail:
                # during the stream phase the DMA device has zero spare
                # bandwidth (any detour starves the PE and resets its
                # p-state ramp), while after the last slab it idles.  A
                # 1-element SBUF->SBUF copy reading the last slab blocks the
                # Pool sequencer until the stream is done.
                with tc.tile_wait_until(defer_ms):
                    # prep arrays are staged host-side already in [P, CB]
                    # layout: each load is one contiguous descriptor per
                    # partition instead of a 4-byte-element scatter (which
                    # costs ~1us of DMA-device time each via the small
                    # descriptor penalty)
                    prep_tiles = {}
                    for nm, src, dt in (("idxt", idx, I32),
                                        ("bpt", bpos, I32),
                                        ("g", gh, F32)):
                        t = epi.tile([P, CB], dt, tag=nm, name=nm)
                        nc.gpsimd.dma_start(out=t[:], in_=src[:])
                        prep_tiles[nm] = t
                    idxt = prep_tiles["idxt"]
                    for m in range(CB):
                        emit_u(m, prep_tiles["bpt"], prep_tiles["g"])
                    wgs = []
                    for m in range(CB):
                        wg = epi.tile([P, O], BF16, tag=f"wg{m}", name=f"wg{m}")
                        nc.gpsimd.indirect_dma_start(
                            out=wg[:], out_offset=None,
                            in_=w[:],
                            in_offset=bass.IndirectOffsetOnAxis(
                                ap=idxt[:, m:m + 1], axis=0),
                        )
                        wgs.append(wg)
                    for m in range(CB):
                        corr = epi.tile([P, O], BF16, tag=f"corr{m}",
                                        name=f"corr{m}")
                        nc.vector.tensor_scalar(
                            corr[:], wgs[m][:], us[m][:, :1], None,
                            mybir.AluOpType.mult,
                        )
                        nc.gpsimd.dma_start(
                            out=corr_o[m * P:(m + 1) * P, :], in_=corr[:])
                tc.tile_update_base_wait()

                # epilogue: per O-half, drain the 4 PSUM banks into one SBUF
                # tile (copies split across DVE and ACT so they pipeline
                # behind the stop-matmuls); each half written with `osplit`
                # DMAs so the device can start before all copies land
                OSPLIT = 2
                for h in range(O // P):
                    outt = epi.tile([P, BC], BF16, tag=f"outt{h}",
                                    name=f"outt{h}")
                    for j in range(NJ):
                        dst = outt[:, j * 512:(j + 1) * 512]
                        if j % 2 == 0:
                            nc.vector.tensor_copy(out=dst, in_=psums[h][j][:])
                        else:
                            nc.scalar.copy(out=dst, in_=psums[h][j][:])
                        done = (j + 1) * 512
                        edge = done * OSPLIT // BC
                        prev_edge = j * 512 * OSPLIT // BC
                        if edge > prev_edge:
                            c0 = (edge - 1) * (BC // OSPLIT)
                            c1 = edge * (BC // OSPLIT)
                            eng = nc.sync if h % 2 == 0 else nc.scalar
                            eng.dma_start(
                                out=out[h * P:(h + 1) * P, c0:c1],
                                in_=outt[:, c0:c1])

    _split_multi_waits(nc)
    return nc


_NC_CACHE = {}


def _get_nc(reps=1, with_bias=False):
    key = (reps, with_bias)
    if key not in _NC_CACHE:
        _NC_CACHE[key] = build(reps, with_bias=with_bias)
    return _NC_CACHE[key]


def make_in_maps(x, W, b, bitswap_coeff, idx, bit_positions):
    import ml_dtypes
    bf16 = ml_dtypes.bfloat16
    x = np.asarray(x, dtype=np.float32)
    Wbf = np.ascontiguousarray(np.asarray(W, dtype=np.float32).astype(bf16))
    coeff = np.full((P, 1), np.asarray(bitswap_coeff, dtype=np.float32))
    idx = np.asarray(idx, dtype=np.int32)
    bpos = np.asarray(bit_positions, dtype=np.int32)
    g_all = x[np.arange(B), idx].astype(np.float32)
    xbf = x.astype(bf16)
    in_maps = []
    for c in range(N_CORES):
        h, f = divmod(c, FS)
        rows = slice(h * BC, (h + 1) * BC)
        cols = slice(f * Fs, (f + 1) * Fs)
        crows = slice(h * BC + f * CR, h * BC + (f + 1) * CR)
        in_maps.append({
            "xt": np.ascontiguousarray(xbf[rows, cols].T),
            "w": Wbf,
            "wsh": np.ascontiguousarray(Wbf[cols]),
            # [P, CB] layout: element (p, m) = value at row m*P + p
            "gh": np.ascontiguousarray(g_all[crows].reshape(CB, P).T),
            "idx": np.ascontiguousarray(idx[crows].reshape(CB, P).T),
            "bpos": np.ascontiguousarray(bpos[crows].reshape(CB, P).T),
            "coeff": coeff,
        })
    return in_maps


def kernel(x, W, b, bitswap_coeff, idx, bit_positions):
    nc = _get_nc()
    in_maps = make_in_maps(x, W, b, bitswap_coeff, idx, bit_positions)
    res = run_bass_kernel_spmd(nc, in_maps, core_ids=list(range(N_CORES)))
    out = np.empty((B, O), dtype=np.float32)
    for h in range(BH):
        acc = res.results[h * FS]["out"].astype(np.float32)
        for f in range(1, FS):
            acc += res.results[h * FS + f]["out"].astype(np.float32)
        out[h * BC:(h + 1) * BC] = acc.T
    corr = np.concatenate(
        [res.results[c]["corr"] for c in range(N_CORES)], axis=0)
    out += corr.astype(np.float32)
    out += np.asarray(b, dtype=np.float32)[None, :]
    return out


# revision 46
# speedup vs baseline: 1.0008x; 1.0008x over previous
"""Trainium2 Bass kernel for nn_BitSwapWrapper.

Reference computation:
    g    = x[rows, idx]                       # one gathered element per row
    u    = coeff * (bitflip(g, bit_pos) - g)
    pert = scatter(zeros_like(x), (rows, idx), u)
    out  = (x + pert) @ W + b

Because pert has exactly one nonzero per row, (x + pert) @ W decomposes as
    out[i, :] = (x @ W)[i, :] + u[i] * W[idx[i], :] + b
so no [B, F] scatter tensor is ever materialized: the kernel streams x
through a K-accumulated matmul and applies the rank-per-row correction with
an indirect-DMA gather of the needed W rows.

Distribution: 2D sharding — batch split in 2 halves x contraction (F) split
in 4 shards.  Core c = h*4 + f streams x[half h, shard f].T @ W[shard f]
into PSUM (bf16 operands, fp32 accumulate, W-stationary so the moving
operand is the 2048-wide batch), writes its partial [O, 2048] in bf16, and
separately computes the scatter correction u[i] * W[idx[i], :] for its own
512-row slice of the batch (exact fp32 u from host-gathered g, indirect-DMA
gather of W rows).  The host sums the 4 partials per batch half (fp32),
adds the per-row corrections at their global rows, and adds the bias.

bf16 operands halve the dominant HBM traffic vs fp32/fp32r; splitting W
over 4 cores cuts the replicated-W read 4x.  Per-core HBM bytes:
16.8 MB (x) + 2.1 MB (W shard) + 1.0 MB (partial out) + 0.6 MB (gathers +
corr) ~= 20.5 MB, vs 48.5 MB for pure data-parallel fp32, putting the
steady-state per-invocation cost at the ~358 GB/s HBM-per-core roofline
(~57 us).  Scheduling details that matter: single-chunk DMA slabs keep the
tensor engine continuously fed (its p-state ramp rewards gap-free
execution); the first chunk arrives piecewise partly via the SWDGE ring so
the PE starts ~1.5 us earlier; the first streamed chunk's matmuls are split
N=128 so the clock ramp is crossed on cheap instructions; correction
gathers/writes are deferred toward the tail where the DMA device idles; and
the epilogue drains each PSUM bank through DVE/ACT copies into split
output DMAs that pipeline behind the stop-matmuls.
"""

import numpy as np

import concourse.bass as bass
import concourse.mybir as mybir
from concourse.bass_utils import run_bass_kernel_spmd
from concourse.tile import TileContext

N_CORES = 8
B, F, O = 4096, 16384, 256
BH, FS = 2, 4            # batch-halves x F-shards
BC = B // BH             # 2048 batch rows per core's matmul
Fs = F // FS             # 4096 contraction rows per core
P = 128
KCs = Fs // P            # 32 contraction chunks per core
CR = B // N_CORES        # 512 correction rows per core
CB = CR // P             # 4 correction row-blocks
NJ = BC // 512           # 4 moving-operand column chunks (512 = 1 PSUM bank)

F32 = mybir.dt.float32
BF16 = mybir.dt.bfloat16
I32 = mybir.dt.int32


def _split_multi_waits(nc):
    """This container's walrus build rejects more than one sync-wait command
    per instruction; split extras onto single-wait NOPs on the same engine."""
    cur_bb = nc.cur_bb.bb
    for f in nc.m.functions:
        for bb in f.blocks:
            il = bb.instructions
            i = 0
            while i < len(il):
                ins = il[i]
                si = getattr(ins, "sync_info", None)
                if si is not None and si.on_wait and len(si.on_wait) > 1:
                    waits = list(si.on_wait)
                    extra, keep = waits[:-1], waits[-1:]
                    carriers = []
                    for w in extra:
                        nop = nc.engines[ins.engine].nop(nofuse=True).ins
                        tail = cur_bb.instructions.pop()
                        assert tail is nop
                        nop.sync_info = mybir.SyncInfo(on_wait=[w], on_update=[])
                        carriers.append(nop)
                    ins.sync_info = mybir.SyncInfo(
                        on_wait=keep, on_update=list(si.on_update or [])
                    )
                    il[i:i] = carriers
                    i += len(carriers)
                i += 1


def build(reps=1, cpg=1, stream_bufs=8, with_bias=False, defer_ms=0.050, wcpg=1, ws0_pool=True, x0_pool=True):
    nc = bass.Bass("TRN2", target_bir_lowering=False, debug=False)
    xt = nc.dram_tensor("xt", [Fs, BC], BF16, kind="ExternalInput").ap()
    w = nc.dram_tensor("w", [F, O], BF16, kind="ExternalInput").ap()
    wsh = nc.dram_tensor("wsh", [Fs, O], BF16, kind="ExternalInput").ap()
    gh = nc.dram_tensor("gh", [P, CB], F32, kind="ExternalInput").ap()
    idx = nc.dram_tensor("idx", [P, CB], I32, kind="ExternalInput").ap()
    bpos = nc.dram_tensor("bpos", [P, CB], I32, kind="ExternalInput").ap()
    coeff = nc.dram_tensor("coeff", [P, 1], F32, kind="ExternalInput").ap()
    out = nc.dram_tensor("out", [O, BC], BF16, kind="ExternalOutput").ap()
    corr_o = nc.dram_tensor("corr", [CR, O], BF16, kind="ExternalOutput").ap()

    with TileContext(nc) as tc:
        with (
            tc.tile_pool(name="stream", bufs=stream_bufs) as stream,
            tc.tile_pool(name="consts", bufs=1) as consts,
            tc.tile_pool(name="epi", bufs=1) as epi,
            tc.tile_pool(name="psum", bufs=1, space="PSUM") as psum,
        ):
            ones_i = consts.tile([P, 1], I32, name="ones_i")
            nc.vector.memset(ones_i[:], 1)
            coeff_b = consts.tile([P, 1], F32, name="coeff_b")
            nc.gpsimd.dma_start(out=coeff_b[:], in_=coeff[:])

            for _ in range(reps):
                psums = [
                    [psum.tile([P, 512], F32, tag=f"ps{h}_{j}", name=f"ps{h}_{j}")
                     for j in range(NJ)]
                    for h in range(O // P)
                ]
                us = []

                def emit_u(m, bpt, g):
                    # u = coeff * (bitflip(g) - g) for row-block m: tiny DVE
                    # work, runs early while the DVE is otherwise idle
                    mask = epi.tile([P, 1], I32, tag=f"mask{m}", name=f"mask{m}")
                    nc.vector.tensor_scalar(
                        mask[:], ones_i[:], bpt[:, m:m + 1], None,
                        mybir.AluOpType.logical_shift_left,
                    )
                    gflip = epi.tile([P, 1], I32, tag=f"gflip{m}", name=f"gflip{m}")
                    nc.vector.tensor_tensor(
                        out=gflip[:], in0=g[:, m:m + 1].bitcast(I32), in1=mask[:],
                        op=mybir.AluOpType.bitwise_xor,
                    )
                    u = epi.tile([P, 1], F32, tag=f"u{m}", name=f"u{m}")
                    nc.vector.tensor_tensor(
                        out=u[:], in0=gflip[:].bitcast(F32), in1=g[:, m:m + 1],
                        op=mybir.AluOpType.subtract,
                    )
                    nc.vector.tensor_tensor(
                        out=u[:], in0=u[:], in1=coeff_b[:],
                        op=mybir.AluOpType.mult,
                    )
                    us.append(u)

                ws0 = epi.tile([P, O], BF16, tag="ws0", name="ws0")
                with tc.high_priority():
                    # the first Ldweights needs ws0; the Pool/SWDGE ring is
                    # idle at program start so it lands ~1.5us earlier than
                    # behind the xs0 pieces on HWDGE
                    eng0 = nc.gpsimd if ws0_pool else nc.scalar
                    eng0.dma_start(out=ws0[:], in_=wsh[0:P, :])
                # chunk 0 arrives as per-bank pieces (finer arrival
                # granularity); optionally the first two ride the SWDGE
                # ring, whose first transfer launches ~1.2us before HWDGE's
                pieces = {}
                for j in range(NJ):
                    tj = stream.tile([P, 512], BF16, tag=f"xp0_{j}",
                                     name=f"xp0_{j}")
                    eng = nc.gpsimd if (x0_pool and j < 2) else nc.sync
                    eng.dma_start(
                        out=tj[:], in_=xt[0:P, j * 512:(j + 1) * 512])
                    pieces[(0, j)] = tj
                for j in range(NJ):
                    for h in range(O // P):
                        nc.tensor.matmul(
                            psums[h][j][:],
                            lhsT=ws0[:, h * P:(h + 1) * P],
                            rhs=pieces[(0, j)][:],
                            start=True,
                            stop=False,
                        )

                CPG = cpg  # k-chunks per DMA slab
                slabs = [(1 + i * CPG, CPG) for i in range((KCs - 1) // CPG)]
                rem = (KCs - 1) % CPG
                if rem:
                    slabs.append((KCs - rem, rem))

                # W chunks stream on the scalar HWDGE ring, wcpg chunks
                # per DMA
                WCPG = wcpg
                ws_of = {}
                wbatches = []
                kw = 1
                while kw < KCs:
                    wn = min(WCPG, KCs - kw)
                    wbatches.append((kw, wn))
                    kw += wn
                for k4, (k0, nch) in enumerate(slabs):
                    r0 = k0 * P
                    xs = stream.tile([P, nch * BC], BF16, tag="xs",
                                     name="xs", padded_shape=[P, CPG * BC])
                    for kw, wn in wbatches:
                        if kw != k0:
                            continue
                        wsb = stream.tile([P, wn * O], BF16, tag="wsb",
                                          name="wsb", padded_shape=[P, WCPG * O])
                        nc.scalar.dma_start(
                            out=wsb[:].rearrange("p (c o) -> p c o", c=wn),
                            in_=wsh[kw * P:(kw + wn) * P, :].rearrange(
                                "(c p) o -> p c o", p=P),
                        )
                        for c in range(wn):
                            ws_of[kw + c] = (wsb, c * O)
                    nc.sync.dma_start(
                        out=xs[:].rearrange("p (c b) -> p c b", c=nch),
                        in_=xt[r0:r0 + nch * P, :].rearrange(
                            "(c p) b -> p c b", p=P),
                    )
                    for c in range(nch):
                        last = k4 == len(slabs) - 1 and c == nch - 1
                        ws, wof = ws_of[k0 + c]
                        # the first streamed chunk accumulates in N=128
                        # pieces: cold-p-state-priced instructions then
                        # cover 1/4 the work while the PE clock ramps
                        # (start=False only accumulates, so safe for PSUM)
                        npc = 4 if k4 == 0 else 1
                        pw = 512 // npc
                        for h in range(O // P):
                            for j in range(NJ):
                                for piece in range(npc):
                                    b0 = c * BC + j * 512 + piece * pw
                                    nc.tensor.matmul(
                                        psums[h][j][:, piece * pw:(piece + 1) * pw],
                                        lhsT=ws[:, wof + h * P:wof + (h + 1) * P],
                                        rhs=xs[:, b0:b0 + pw],
                                        start=False,
                                        stop=last,
                                        skip_group_check=npc > 1,
                                    )
                assert slabs, "need at least one streamed slab"
                # Correction metadata/gather/write work, nudged toward the
                # tail via the scheduler's wait-until hint: during the
                # stream phase the DMA device has zero spare bandwidth (any
                # detour starves the PE and resets its p-state ramp), while
                # after the last slab it idles.
                with tc.tile_wait_until(defer_ms):
                    # prep arrays are staged host-side already in [P, CB]
                    # layout: each load is one contiguous descriptor per
                    # partition instead of a 4-byte-element scatter (which
                    # costs ~1us of DMA-device time each via the small
                    # descriptor penalty)
                    prep_tiles = {}
                    for nm, src, dt in (("idxt", idx, I32),
                                        ("bpt", bpos, I32),
                                        ("g", gh, F32)):
                        t = epi.tile([P, CB], dt, tag=nm, name=nm)
                        nc.gpsimd.dma_start(out=t[:], in_=src[:])
                        prep_tiles[nm] = t
                    idxt = prep_tiles["idxt"]
                    for m in range(CB):
                        emit_u(m, prep_tiles["bpt"], prep_tiles["g"])
                    wgs = []
                    for m in range(CB):
                        wg = epi.tile([P, O], BF16, tag=f"wg{m}", name=f"wg{m}")
                        nc.gpsimd.indirect_dma_start(
                            out=wg[:], out_offset=None,
                            in_=w[:],
                            in_offset=bass.IndirectOffsetOnAxis(
                                ap=idxt[:, m:m + 1], axis=0),
                        )
                        wgs.append(wg)
                    for m in range(CB):
                        corr = epi.tile([P, O], BF16, tag=f"corr{m}",
                                        name=f"corr{m}")
                        nc.vector.tensor_scalar(
                            corr[:], wgs[m][:], us[m][:, :1], None,
                            mybir.AluOpType.mult,
                        )
                        nc.gpsimd.dma_start(
                            out=corr_o[m * P:(m + 1) * P, :], in_=corr[:])
                tc.tile_update_base_wait()

                # epilogue: per O-half, drain the 4 PSUM banks into one SBUF
                # tile (copies split across DVE and ACT so they pipeline
                # behind the stop-matmuls); each half written with OSPLIT
                # DMAs so the device can start before all copies land
                OSPLIT = 2
                for h in range(O // P):
                    outt = epi.tile([P, BC], BF16, tag=f"outt{h}",
                                    name=f"outt{h}")
                    for j in range(NJ):
                        dst = outt[:, j * 512:(j + 1) * 512]
                        if j % 2 == 0:
                            nc.vector.tensor_copy(out=dst, in_=psums[h][j][:])
                        else:
                            nc.scalar.copy(out=dst, in_=psums[h][j][:])
                        done = (j + 1) * 512
                        edge = done * OSPLIT // BC
                        prev_edge = j * 512 * OSPLIT // BC
                        if edge > prev_edge:
                            c0 = (edge - 1) * (BC // OSPLIT)
                            c1 = edge * (BC // OSPLIT)
                            eng = nc.sync if h % 2 == 0 else nc.scalar
                            eng.dma_start(
                                out=out[h * P:(h + 1) * P, c0:c1],
                                in_=outt[:, c0:c1])

    _split_multi_waits(nc)
    return nc


_NC_CACHE = {}


def _get_nc(reps=1, with_bias=False):
    key = (reps, with_bias)
    if key not in _NC_CACHE:
        _NC_CACHE[key] = build(reps, with_bias=with_bias)
    return _NC_CACHE[key]


def make_in_maps(x, W, b, bitswap_coeff, idx, bit_positions):
    import ml_dtypes
    bf16 = ml_dtypes.bfloat16
    x = np.asarray(x, dtype=np.float32)
    Wbf = np.ascontiguousarray(np.asarray(W, dtype=np.float32).astype(bf16))
    coeff = np.full((P, 1), np.asarray(bitswap_coeff, dtype=np.float32))
    idx = np.asarray(idx, dtype=np.int32)
    bpos = np.asarray(bit_positions, dtype=np.int32)
    g_all = x[np.arange(B), idx].astype(np.float32)
    xbf = x.astype(bf16)
    in_maps = []
    for c in range(N_CORES):
        h, f = divmod(c, FS)
        rows = slice(h * BC, (h + 1) * BC)
        cols = slice(f * Fs, (f + 1) * Fs)
        crows = slice(h * BC + f * CR, h * BC + (f + 1) * CR)
        in_maps.append({
            "xt": np.ascontiguousarray(xbf[rows, cols].T),
            "w": Wbf,
            "wsh": np.ascontiguousarray(Wbf[cols]),
            # [P, CB] layout: element (p, m) = value at row m*P + p
            "gh": np.ascontiguousarray(g_all[crows].reshape(CB, P).T),
            "idx": np.ascontiguousarray(idx[crows].reshape(CB, P).T),
            "bpos": np.ascontiguousarray(bpos[crows].reshape(CB, P).T),
            "coeff": coeff,
        })
    return in_maps


def kernel(x, W, b, bitswap_coeff, idx, bit_positions):
    nc = _get_nc()
    in_maps = make_in_maps(x, W, b, bitswap_coeff, idx, bit_positions)
    res = run_bass_kernel_spmd(nc, in_maps, core_ids=list(range(N_CORES)))
    out = np.empty((B, O), dtype=np.float32)
    for h in range(BH):
        acc = res.results[h * FS]["out"].astype(np.float32)
        for f in range(1, FS):
            acc += res.results[h * FS + f]["out"].astype(np.float32)
        out[h * BC:(h + 1) * BC] = acc.T
    corr = np.concatenate(
        [res.results[c]["corr"] for c in range(N_CORES)], axis=0)
    out += corr.astype(np.float32)
    out += np.asarray(b, dtype=np.float32)[None, :]
    return out


# revision 50
# speedup vs baseline: 1.1092x; 1.1083x over previous
"""Trainium2 Bass kernel for nn_BitSwapWrapper.

Reference computation:
    g    = x[rows, idx]                       # one gathered element per row
    u    = coeff * (bitflip(g, bit_pos) - g)
    pert = scatter(zeros_like(x), (rows, idx), u)
    out  = (x + pert) @ W + b

Because pert has exactly one nonzero per row, (x + pert) @ W decomposes as
    out[i, :] = (x @ W)[i, :] + u[i] * W[idx[i], :] + b
so no [B, F] scatter tensor is ever materialized: the kernel streams x
through a K-accumulated matmul and applies the rank-per-row correction with
an indirect-DMA gather of the needed W rows.

Distribution: 2D sharding — batch split in 2 halves x contraction (F) split
in 4 shards.  Core c = h*4 + f streams x[half h, shard f].T @ W[shard f]
into PSUM (bf16 operands, fp32 accumulate, W-stationary so the moving
operand is the 2048-wide batch), writes its partial [O, 2048] in bf16, and
separately computes the scatter correction u[i] * W[idx[i], :] for its own
512-row slice of the batch (exact fp32 u from host-gathered g, indirect-DMA
gather of W rows).  The host sums the 4 partials per batch half (fp32),
adds the per-row corrections at their global rows, and adds the bias.

bf16 operands halve the dominant HBM traffic vs fp32/fp32r; splitting W
over 4 cores cuts the replicated-W read 4x.  Per-core HBM bytes:
16.8 MB (x) + 2.1 MB (W shard) + 1.0 MB (partial out) + 0.6 MB (gathers +
corr) ~= 20.5 MB, vs 48.5 MB for pure data-parallel fp32, putting the
steady-state per-invocation cost at the ~358 GB/s HBM-per-core roofline
(~57 us).  Scheduling details that matter: small DMA slabs keep the
tensor engine continuously fed (its p-state ramp rewards gap-free
execution); the first chunk arrives piecewise partly via the SWDGE ring so
the PE starts ~1.5 us earlier; the first streamed chunk's matmuls are split
N=128 so the clock ramp is crossed on cheap instructions; correction
metadata rides one merged [P, 3*CB] load and one merged correction store,
deferred toward the tail where the DMA device idles; and the epilogue
drains each PSUM bank through DVE/ACT copies into split output DMAs that
pipeline behind the stop-matmuls.  DMA count per rep is kept low (~40) --
real-HW per-transfer overhead sits above the cost model's.
"""

import numpy as np

import concourse.bass as bass
import concourse.mybir as mybir
from concourse.bass_utils import run_bass_kernel_spmd
from concourse.tile import TileContext

N_CORES = 8
B, F, O = 4096, 16384, 256
BH, FS = 2, 4            # batch-halves x F-shards
BC = B // BH             # 2048 batch rows per core's matmul
Fs = F // FS             # 4096 contraction rows per core
P = 128
KCs = Fs // P            # 32 contraction chunks per core
CR = B // N_CORES        # 512 correction rows per core
CB = CR // P             # 4 correction row-blocks
NJ = BC // 512           # 4 moving-operand column chunks (512 = 1 PSUM bank)

F32 = mybir.dt.float32
BF16 = mybir.dt.bfloat16
I32 = mybir.dt.int32


def _split_multi_waits(nc):
    """This container's walrus build rejects more than one sync-wait command
    per instruction; split extras onto single-wait NOPs on the same engine."""
    cur_bb = nc.cur_bb.bb
    for f in nc.m.functions:
        for bb in f.blocks:
            il = bb.instructions
            i = 0
            while i < len(il):
                ins = il[i]
                si = getattr(ins, "sync_info", None)
                if si is not None and si.on_wait and len(si.on_wait) > 1:
                    waits = list(si.on_wait)
                    extra, keep = waits[:-1], waits[-1:]
                    carriers = []
                    for w in extra:
                        nop = nc.engines[ins.engine].nop(nofuse=True).ins
                        tail = cur_bb.instructions.pop()
                        assert tail is nop
                        nop.sync_info = mybir.SyncInfo(on_wait=[w], on_update=[])
                        carriers.append(nop)
                    ins.sync_info = mybir.SyncInfo(
                        on_wait=keep, on_update=list(si.on_update or [])
                    )
                    il[i:i] = carriers
                    i += len(carriers)
                i += 1


def build(reps=1, cpg=2, stream_bufs=6, with_bias=False, defer_ms=0.050, wcpg=4, ws0_pool=True, x0_pool=True):
    nc = bass.Bass("TRN2", target_bir_lowering=False, debug=False)
    xt = nc.dram_tensor("xt", [Fs, BC], BF16, kind="ExternalInput").ap()
    w = nc.dram_tensor("w", [F, O], BF16, kind="ExternalInput").ap()
    wsh = nc.dram_tensor("wsh", [Fs, O], BF16, kind="ExternalInput").ap()
    # idx | bpos | gh-bits concatenated: one load instead of three
    prep = nc.dram_tensor("prep", [P, 3 * CB], I32, kind="ExternalInput").ap()
    coeff = nc.dram_tensor("coeff", [P, 1], F32, kind="ExternalInput").ap()
    out = nc.dram_tensor("out", [O, BC], BF16, kind="ExternalOutput").ap()
    corr_o = nc.dram_tensor("corr", [CR, O], BF16, kind="ExternalOutput").ap()

    with TileContext(nc) as tc:
        with (
            tc.tile_pool(name="stream", bufs=stream_bufs) as stream,
            tc.tile_pool(name="consts", bufs=1) as consts,
            tc.tile_pool(name="epi", bufs=1) as epi,
            tc.tile_pool(name="psum", bufs=1, space="PSUM") as psum,
        ):
            ones_i = consts.tile([P, 1], I32, name="ones_i")
            nc.vector.memset(ones_i[:], 1)
            coeff_b = consts.tile([P, 1], F32, name="coeff_b")
            nc.gpsimd.dma_start(out=coeff_b[:], in_=coeff[:])

            for _ in range(reps):
                psums = [
                    [psum.tile([P, 512], F32, tag=f"ps{h}_{j}", name=f"ps{h}_{j}")
                     for j in range(NJ)]
                    for h in range(O // P)
                ]
                us = []

                def emit_u(m, bpt, g):
                    # u = coeff * (bitflip(g) - g) for row-block m: tiny DVE
                    # work, runs early while the DVE is otherwise idle
                    mask = epi.tile([P, 1], I32, tag=f"mask{m}", name=f"mask{m}")
                    nc.vector.tensor_scalar(
                        mask[:], ones_i[:], bpt[:, m:m + 1], None,
                        mybir.AluOpType.logical_shift_left,
                    )
                    gflip = epi.tile([P, 1], I32, tag=f"gflip{m}", name=f"gflip{m}")
                    nc.vector.tensor_tensor(
                        out=gflip[:], in0=g[:, m:m + 1].bitcast(I32), in1=mask[:],
                        op=mybir.AluOpType.bitwise_xor,
                    )
                    u = epi.tile([P, 1], F32, tag=f"u{m}", name=f"u{m}")
                    nc.vector.tensor_tensor(
                        out=u[:], in0=gflip[:].bitcast(F32), in1=g[:, m:m + 1],
                        op=mybir.AluOpType.subtract,
                    )
                    nc.vector.tensor_tensor(
                        out=u[:], in0=u[:], in1=coeff_b[:],
                        op=mybir.AluOpType.mult,
                    )
                    us.append(u)

                ws0 = epi.tile([P, O], BF16, tag="ws0", name="ws0")
                with tc.high_priority():
                    # the first Ldweights needs ws0; the Pool/SWDGE ring is
                    # idle at program start so it lands ~1.5us earlier than
                    # behind the xs0 pieces on HWDGE
                    eng0 = nc.gpsimd if ws0_pool else nc.scalar
                    eng0.dma_start(out=ws0[:], in_=wsh[0:P, :])
                # chunk 0 arrives as per-bank pieces (finer arrival
                # granularity); optionally the first two ride the SWDGE
                # ring, whose first transfer launches ~1.2us before HWDGE's
                pieces = {}
                for j in range(NJ):
                    tj = stream.tile([P, 512], BF16, tag=f"xp0_{j}",
                                     name=f"xp0_{j}")
                    eng = nc.gpsimd if (x0_pool and j < 2) else nc.sync
                    eng.dma_start(
                        out=tj[:], in_=xt[0:P, j * 512:(j + 1) * 512])
                    pieces[(0, j)] = tj
                for j in range(NJ):
                    for h in range(O // P):
                        nc.tensor.matmul(
                            psums[h][j][:],
                            lhsT=ws0[:, h * P:(h + 1) * P],
                            rhs=pieces[(0, j)][:],
                            start=True,
                            stop=False,
                        )

                CPG = cpg  # k-chunks per DMA slab
                slabs = [(1 + i * CPG, CPG) for i in range((KCs - 1) // CPG)]
                rem = (KCs - 1) % CPG
                if rem:
                    slabs.append((KCs - rem, rem))

                # W chunks stream on the scalar HWDGE ring, wcpg chunks
                # per DMA
                WCPG = wcpg
                ws_of = {}
                wbatches = []
                kw = 1
                while kw < KCs:
                    wn = min(WCPG, KCs - kw)
                    wbatches.append((kw, wn))
                    kw += wn
                for k4, (k0, nch) in enumerate(slabs):
                    r0 = k0 * P
                    xs = stream.tile([P, nch * BC], BF16, tag="xs",
                                     name="xs", padded_shape=[P, CPG * BC])
                    for kw, wn in wbatches:
                        if kw != k0:
                            continue
                        wsb = stream.tile([P, wn * O], BF16, tag="wsb",
                                          name="wsb", padded_shape=[P, WCPG * O])
                        nc.scalar.dma_start(
                            out=wsb[:].rearrange("p (c o) -> p c o", c=wn),
                            in_=wsh[kw * P:(kw + wn) * P, :].rearrange(
                                "(c p) o -> p c o", p=P),
                        )
                        for c in range(wn):
                            ws_of[kw + c] = (wsb, c * O)
                    nc.sync.dma_start(
                        out=xs[:].rearrange("p (c b) -> p c b", c=nch),
                        in_=xt[r0:r0 + nch * P, :].rearrange(
                            "(c p) b -> p c b", p=P),
                    )
                    for c in range(nch):
                        last = k4 == len(slabs) - 1 and c == nch - 1
                        ws, wof = ws_of[k0 + c]
                        # the first streamed chunk accumulates in N=128
                        # pieces: cold-p-state-priced instructions then
                        # cover 1/4 the work while the PE clock ramps
                        # (start=False only accumulates, so safe for PSUM)
                        npc = 4 if k4 == 0 else 1
                        pw = 512 // npc
                        for h in range(O // P):
                            for j in range(NJ):
                                for piece in range(npc):
                                    b0 = c * BC + j * 512 + piece * pw
                                    nc.tensor.matmul(
                                        psums[h][j][:, piece * pw:(piece + 1) * pw],
                                        lhsT=ws[:, wof + h * P:wof + (h + 1) * P],
                                        rhs=xs[:, b0:b0 + pw],
                                        start=False,
                                        stop=last,
                                        skip_group_check=npc > 1,
                                    )
                assert slabs, "need at least one streamed slab"
                # Correction metadata/gather/write work, nudged toward the
                # tail via the scheduler's wait-until hint: during the
                # stream phase the DMA device has zero spare bandwidth (any
                # detour starves the PE and resets its p-state ramp), while
                # after the last slab it idles.
                with tc.tile_wait_until(defer_ms):
                    # prep arrays are staged host-side already in [P, CB]
                    # layout: each row of the staged array is contiguous per
                    # partition (a 4-byte-element scatter would cost ~1us of
                    # DMA-device time via the small-descriptor penalty), and
                    # idx/bpos/g ride one DMA instead of three
                    pt = epi.tile([P, 3 * CB], I32, tag="prep", name="prep")
                    nc.gpsimd.dma_start(out=pt[:], in_=prep[:])
                    idxt = pt[:, 0:CB]
                    for m in range(CB):
                        emit_u(m, pt[:, CB:2 * CB],
                               pt[:, 2 * CB:3 * CB].bitcast(F32))
                    wgs = []
                    for m in range(CB):
                        wg = epi.tile([P, O], BF16, tag=f"wg{m}", name=f"wg{m}")
                        nc.gpsimd.indirect_dma_start(
                            out=wg[:], out_offset=None,
                            in_=w[:],
                            in_offset=bass.IndirectOffsetOnAxis(
                                ap=idxt[:, m:m + 1], axis=0),
                        )
                        wgs.append(wg)
                    # all 4 row-blocks' corrections in one tile -> one DMA
                    corr = epi.tile([P, CB * O], BF16, tag="corr", name="corr")
                    for m in range(CB):
                        nc.vector.tensor_scalar(
                            corr[:, m * O:(m + 1) * O], wgs[m][:],
                            us[m][:, :1], None, mybir.AluOpType.mult,
                        )
                    nc.gpsimd.dma_start(
                        out=corr_o.rearrange("(m p) o -> p m o", p=P),
                        in_=corr[:].rearrange("p (m o) -> p m o", m=CB))
                tc.tile_update_base_wait()

                # epilogue: per O-half, drain the 4 PSUM banks into one SBUF
                # tile (copies split across DVE and ACT so they pipeline
                # behind the stop-matmuls); each half written with OSPLIT
                # DMAs so the device can start before all copies land
                OSPLIT = 2
                for h in range(O // P):
                    outt = epi.tile([P, BC], BF16, tag=f"outt{h}",
                                    name=f"outt{h}")
                    for j in range(NJ):
                        dst = outt[:, j * 512:(j + 1) * 512]
                        if j % 2 == 0:
                            nc.vector.tensor_copy(out=dst, in_=psums[h][j][:])
                        else:
                            nc.scalar.copy(out=dst, in_=psums[h][j][:])
                        done = (j + 1) * 512
                        edge = done * OSPLIT // BC
                        prev_edge = j * 512 * OSPLIT // BC
                        if edge > prev_edge:
                            c0 = (edge - 1) * (BC // OSPLIT)
                            c1 = edge * (BC // OSPLIT)
                            eng = nc.sync if h % 2 == 0 else nc.scalar
                            eng.dma_start(
                                out=out[h * P:(h + 1) * P, c0:c1],
                                in_=outt[:, c0:c1])

    _split_multi_waits(nc)
    return nc


_NC_CACHE = {}


def _get_nc(reps=1, with_bias=False):
    key = (reps, with_bias)
    if key not in _NC_CACHE:
        _NC_CACHE[key] = build(reps, with_bias=with_bias)
    return _NC_CACHE[key]


def make_in_maps(x, W, b, bitswap_coeff, idx, bit_positions):
    import ml_dtypes
    bf16 = ml_dtypes.bfloat16
    x = np.asarray(x, dtype=np.float32)
    Wbf = np.ascontiguousarray(np.asarray(W, dtype=np.float32).astype(bf16))
    coeff = np.full((P, 1), np.asarray(bitswap_coeff, dtype=np.float32))
    idx = np.asarray(idx, dtype=np.int32)
    bpos = np.asarray(bit_positions, dtype=np.int32)
    g_all = x[np.arange(B), idx].astype(np.float32)
    xbf = x.astype(bf16)
    in_maps = []
    for c in range(N_CORES):
        h, f = divmod(c, FS)
        rows = slice(h * BC, (h + 1) * BC)
        cols = slice(f * Fs, (f + 1) * Fs)
        crows = slice(h * BC + f * CR, h * BC + (f + 1) * CR)
        in_maps.append({
            "xt": np.ascontiguousarray(xbf[rows, cols].T),
            "w": Wbf,
            "wsh": np.ascontiguousarray(Wbf[cols]),
            # [P, 3*CB] i32: idx | bpos | gh-bits, element (p, m) = value
            # at correction row m*P + p
            "prep": np.ascontiguousarray(np.concatenate([
                idx[crows].reshape(CB, P).T,
                bpos[crows].reshape(CB, P).T,
                g_all[crows].reshape(CB, P).T.view(np.int32),
            ], axis=1)),
            "coeff": coeff,
        })
    return in_maps


def kernel(x, W, b, bitswap_coeff, idx, bit_positions):
    nc = _get_nc()
    in_maps = make_in_maps(x, W, b, bitswap_coeff, idx, bit_positions)
    res = run_bass_kernel_spmd(nc, in_maps, core_ids=list(range(N_CORES)))
    out = np.empty((B, O), dtype=np.float32)
    for h in range(BH):
        acc = res.results[h * FS]["out"].astype(np.float32)
        for f in range(1, FS):
            acc += res.results[h * FS + f]["out"].astype(np.float32)
        out[h * BC:(h + 1) * BC] = acc.T
    corr = np.concatenate(
        [res.results[c]["corr"] for c in range(N_CORES)], axis=0)
    out += corr.astype(np.float32)
    out += np.asarray(b, dtype=np.float32)[None, :]
    return out
